# revision 3
# baseline (speedup 1.0000x reference)
"""Trainium2 Bass kernel for the SOCNet battery state-of-charge model.

Math (per battery cell b, timestep t):
    h   = softplus(w0*I + w1*Temp + b1e)
    f   = eta0*(1 + w2e*h + b2e) * I / (3600*Q)
    out[b, 0] = SOC_init(b)          (tiny net on first-timestep features)
    out[b, t] = SOC_init(b) + sum_{j<t} (ts[j+1]-ts[j]) * f[j]

Strategy: pure data parallel over 8 NeuronCores (128 batch rows per core =
128 SBUF partitions).  The tiny per-cell scalars (SOC_init, q1, q2) are
precomputed on host; the [128, 8192] heavy math runs on DVE/ACT with the
cumulative sum done by the DVE's native tensor_tensor_scan, chained across
T-chunks via a carry column.
"""

import numpy as np

B, T, F = 1024, 8192, 4
NCORES = 8
BS = B // NCORES  # 128 rows per core == SBUF partition count
TC = 2048         # timesteps per chunk


def _softplus64(x):
    x = x.astype(np.float64)
    return np.logaddexp(0.0, x)


def _build_program(k_piv, piv_col, oth_col, act_scale, reps=1):
    from contextlib import ExitStack

    import bass_rust as _bass_rust
    import concourse.bass as bass
    import concourse.mybir as mybir
    import concourse.tile as tile

    f32 = mybir.dt.float32
    nc = bass.Bass()

    xd = nc.dram_tensor("x", [BS, T * F], f32, kind="ExternalInput")
    pd = nc.dram_tensor("p", [BS, 4], f32, kind="ExternalInput")
    od = nc.dram_tensor("o", [BS, T], f32, kind="ExternalOutput")

    with ExitStack() as ctx:
        tc = ctx.enter_context(tile.TileContext(nc))
        xpool = ctx.enter_context(tc.tile_pool(name="x", bufs=3))
        wpool = ctx.enter_context(tc.tile_pool(name="w", bufs=2))
        epool = ctx.enter_context(tc.tile_pool(name="e", bufs=2))
        rpool = ctx.enter_context(tc.tile_pool(name="r", bufs=3))
        cpool = ctx.enter_context(tc.tile_pool(name="c", bufs=1))

        ones = cpool.tile([BS, TC], f32)
        nc.vector.memset(ones[:], 1.0)
        ptile = cpool.tile([BS, 4], f32)
        nc.sync.dma_start(ptile[:], pd[:])
        # DVE-made copy of the per-cell scalars: the activations' bias/scale
        # reads then depend only on the DVE semaphore (the Activation ISA
        # struct has a single sync-wait slot, and every activation here
        # already waits on a DVE-produced input).
        pact = cpool.tile([BS, 4], f32)
        nc.vector.tensor_copy(pact[:], ptile[:])
        # out column 0 is SOC_init itself
        nc.gpsimd.dma_start(od[:, 0:1], ptile[:, 0:1])

        sizes = []
        rem = T - 1
        while rem > 0:
            sizes.append(min(TC, rem))
            rem -= sizes[-1]

        for _rep in range(reps):
            carry = ptile[:, 0:1]
            s = 0
            for L in sizes:
                xt = xpool.tile([BS, (TC + 1) * F], f32)
                nc.sync.dma_start(
                    xt[:, : (L + 1) * F], xd[:, s * F : (s + L + 1) * F]
                )
                x3 = xt[:].rearrange("p (t f) -> p t f", f=F)

                ts0 = x3[:, 0:L, 0]
                ts1 = x3[:, 1 : L + 1, 0]
                cur = x3[:, 0:L, 1]
                piv = x3[:, 0:L, piv_col]
                oth = x3[:, 0:L, oth_col]
                dt = x3[:, 0:L, 3]  # unused U column reused as scratch

                wt = wpool.tile([BS, TC], f32)
                # wt = piv*k + oth   (the softplus pre-activation, un-scaled)
                nc.vector.scalar_tensor_tensor(
                    wt[:, :L], piv, float(k_piv), oth,
                    mybir.AluOpType.mult, mybir.AluOpType.add,
                )
                # wt = softplus(act_scale*wt + act_bias) = ln(1 + exp(.))
                # (the toolchain's ACT func sets have exp+ln+identity together;
                #  native Softplus fails to lower)
                nc.scalar.activation(
                    wt[:, :L], wt[:, :L], mybir.ActivationFunctionType.Exp,
                    bias=pact[:, 3:4], scale=float(act_scale),
                )
                nc.scalar.activation(
                    wt[:, :L], wt[:, :L], mybir.ActivationFunctionType.Ln,
                    bias=1.0, scale=1.0,
                )
                # wt = q2*wt + q1    (per-cell scalars)
                nc.scalar.activation(
                    wt[:, :L], wt[:, :L], mybir.ActivationFunctionType.Identity,
                    bias=pact[:, 1:2], scale=pact[:, 2:3],
                )
                # dt = ts[t+1] - ts[t]
                nc.vector.tensor_sub(dt, ts1, ts0)
                et = epool.tile([BS, TC], f32)
                nc.vector.tensor_mul(et[:, :L], cur, dt)
                nc.vector.tensor_mul(et[:, :L], et[:, :L], wt[:, :L])
                # running SOC: r[i] = carry + cumsum(incr)[i]
                rt = rpool.tile([BS, TC], f32)
                nc.vector.tensor_tensor_scan(
                    rt[:, :L], ones[:, :L], et[:, :L], carry,
                    mybir.AluOpType.mult, mybir.AluOpType.add,
                )
                nc.gpsimd.dma_start(od[:, s + 1 : s + L + 1], rt[:, :L])
                carry = rt[:, L - 1 : L]
                s += L

    # neuronxcc codegen allows at most one sync wait per instruction; split
    # multi-wait instructions the way Bacc.compile() would.
    _bass_rust.generate_event_semaphores(nc)
    return nc


def _prep(X, SC, W1i, b1i, W2i, b2i, W1e, b1e, W2e, b2e):
    """Host precompute: returns (build_params, in_maps) where
    build_params = (k_piv, piv_col, oth_col, act_scale) for _build_program
    and in_maps is the per-core input dict list."""
    X = np.ascontiguousarray(np.asarray(X), dtype=np.float32)
    SC = np.ascontiguousarray(np.asarray(SC), dtype=np.float32)
    W1i = np.asarray(W1i, dtype=np.float64)
    b1i = np.asarray(b1i, dtype=np.float64)
    W2i = np.asarray(W2i, dtype=np.float64)
    b2i = np.asarray(b2i, dtype=np.float64)
    W1e = np.asarray(W1e, dtype=np.float64)
    b1e = np.asarray(b1e, dtype=np.float64)
    W2e = np.asarray(W2e, dtype=np.float64)
    b2e = np.asarray(b2e, dtype=np.float64)

    # ---- host precompute of tiny per-cell scalars (float64 for accuracy) ----
    Q = SC[:, 0].astype(np.float64)
    eta0 = SC[:, 1].astype(np.float64)
    R = SC[:, 2].astype(np.float64)
    soc_base = SC[:, 3].astype(np.float64)

    feat0 = np.stack(
        [X[:, 0, 1], X[:, 0, 2], X[:, 0, 3], SC[:, 2]], axis=-1
    ).astype(np.float64)  # [B, 4] = (I0, Temp0, U0, R)
    z = feat0 @ W1i.T + b1i
    h0 = _softplus64(z)
    soc_net = (h0 @ W2i.T + b2i)[:, 0]
    soc_init = soc_base * (1.0 + soc_net)  # [B]

    c = eta0 / (3600.0 * Q)
    b2e_f = float(np.asarray(b2e).reshape(-1)[0])
    w2e_f = float(np.asarray(W2e).reshape(-1)[0])
    q1 = c * (1.0 + b2e_f)  # [B]
    q2 = c * w2e_f          # [B]

    # pre-activation a = w0*I + w1*Temp + b1e, computed as
    # act_scale*(piv*k + oth) + act_bias with the larger weight as pivot
    w0 = float(np.asarray(W1e).reshape(-1)[0])
    w1 = float(np.asarray(W1e).reshape(-1)[1])
    b1e_f = float(np.asarray(b1e).reshape(-1)[0])
    if abs(w0) >= abs(w1):
        # a = w0*(I + (w1/w0)*Temp) + b  -> pivot=Temp(col2), other=I(col1)
        piv_col, oth_col = 2, 1
        k_piv = w1 / w0 if w0 != 0.0 else 0.0
        act_scale = w0
    else:
        piv_col, oth_col = 1, 2
        k_piv = w0 / w1
        act_scale = w1

    P = np.stack(
        [soc_init, q1, q2, np.full_like(q1, b1e_f)], axis=-1
    ).astype(np.float32)  # [B, 4]

    in_maps = []
    for ci in range(NCORES):
        sl = slice(ci * BS, (ci + 1) * BS)
        in_maps.append(
            {
                "x": np.ascontiguousarray(X[sl]).reshape(BS, T * F),
                "p": np.ascontiguousarray(P[sl]),
            }
        )
    return (k_piv, piv_col, oth_col, act_scale), in_maps


def kernel(X, SC, W1i, b1i, W2i, b2i, W1e, b1e, W2e, b2e):
    from concourse.bass_utils import run_bass_kernel_spmd

    params, in_maps = _prep(X, SC, W1i, b1i, W2i, b2i, W1e, b1e, W2e, b2e)
    nc = _build_program(*params)

    res = run_bass_kernel_spmd(nc, in_maps, list(range(NCORES)))
    out = np.concatenate([res.results[ci]["o"] for ci in range(NCORES)], axis=0)
    return out.reshape(B, T, 1)



# revision 9
# speedup vs baseline: 1.7648x; 1.7648x over previous
"""Trainium2 Bass kernel for the SOCNet battery state-of-charge model.

Math (per battery cell b, timestep t):
    h   = softplus(w0*I + w1*Temp + b1e)
    f   = eta0*(1 + w2e*h + b2e) * I / (3600*Q)
    out[b, 0] = SOC_init(b)          (tiny net on first-timestep features)
    out[b, t] = SOC_init(b) + sum_{j<t} (ts[j+1]-ts[j]) * f[j]

Strategy: pure data parallel over 8 NeuronCores (128 batch rows per core =
128 SBUF partitions).  The tiny per-cell scalars (SOC_init, q1, q2) are
precomputed on host; the [128, 8192] heavy math runs on DVE/ACT with the
cumulative sum done by the DVE's native tensor_tensor_scan, chained across
T-chunks via a carry column.
"""

import numpy as np

B, T, F = 1024, 8192, 4
NCORES = 8
BS = B // NCORES  # 128 rows per core == SBUF partition count
TC = 2048         # timesteps per chunk


def _softplus64(x):
    x = x.astype(np.float64)
    return np.logaddexp(0.0, x)


_SCAN_MUL = None


def _scan_mul_op():
    """Register (once) a fused custom DVE op:
        out[p, k] = s0[p] + sum_{j<=k} in0[p, j] * in1[p, j]
    The stock tensor_tensor_scan routes its recurrence backward through the
    datapath and runs at well under half throughput; a lower()-generated scan
    is a one-cycle recurrence (1 elem/cycle) and also absorbs the multiply,
    replacing two stock DVE passes (tensor_mul + tensor_tensor_scan)."""
    global _SCAN_MUL
    if _SCAN_MUL is not None:
        return _SCAN_MUL
    from concourse import dve_ops
    from concourse.dve_spec import AluOp, C0, Spec, Src0, Src1, _has_src1, lower, scan
    from concourse.dve_uop import DveOpSpec

    name = "SOC_SCAN_MUL"
    if name in dve_ops._SUB_OPCODE_FOR_NAME:
        _SCAN_MUL = next(o for o in dve_ops.OPS if o.name == name)
        return _SCAN_MUL

    spec = Spec(
        body=scan(AluOp.ADD, Src0 * Src1, init=C0),
        reference=lambda in0, in1, s0, s1, imm2: (
            np.asarray(s0, np.float32).reshape(-1, 1)
            + np.cumsum(
                in0.astype(np.float32) * in1.astype(np.float32),
                axis=-1, dtype=np.float32,
            )
        ),
    )
    opcode = dve_ops._CUSTOM_DVE_ROW_BASE + len(dve_ops.OPS)
    shas = {}
    for ver in ("v3", "v4"):
        uops = lower(spec, ver=ver)
        shas[ver] = DveOpSpec(
            name=name, opcode=opcode, uops=uops, rd1_en=_has_src1(spec)
        ).sha(ver)
    op = dve_ops.DveOp(name, spec, subdim=False, uops_sha=shas)
    dve_ops.OPS.append(op)
    dve_ops.CUSTOM_DVE_SPECS[name] = spec
    dve_ops._SUB_OPCODE_FOR_NAME[name] = opcode
    _SCAN_MUL = op
    return op


def _build_program(k_piv, piv_col, oth_col, act_scale, reps=1):
    from contextlib import ExitStack

    import bass_rust as _bass_rust
    import concourse.bass as bass
    import concourse.mybir as mybir
    import concourse.tile as tile

    f32 = mybir.dt.float32
    nc = bass.Bass()

    xd = nc.dram_tensor("x", [BS, T * F], f32, kind="ExternalInput")
    pd = nc.dram_tensor("p", [BS, 4], f32, kind="ExternalInput")
    od = nc.dram_tensor("o", [BS, T], f32, kind="ExternalOutput")

    with ExitStack() as ctx:
        tc = ctx.enter_context(tile.TileContext(nc))
        xpool = ctx.enter_context(tc.tile_pool(name="x", bufs=3))
        wpool = ctx.enter_context(tc.tile_pool(name="w", bufs=2))
        dpool = ctx.enter_context(tc.tile_pool(name="d", bufs=2))
        mpool = ctx.enter_context(tc.tile_pool(name="m", bufs=2))
        epool = ctx.enter_context(tc.tile_pool(name="e", bufs=2))
        rpool = ctx.enter_context(tc.tile_pool(name="r", bufs=3))
        cpool = ctx.enter_context(tc.tile_pool(name="c", bufs=1))

        ones = cpool.tile([BS, TC], f32)
        nc.vector.memset(ones[:], 1.0)
        ptile = cpool.tile([BS, 4], f32)
        nc.sync.dma_start(ptile[:], pd[:])
        # DVE-made copy of the per-cell scalars: the activations' bias/scale
        # reads then depend only on the DVE semaphore (the Activation ISA
        # struct has a single sync-wait slot, and every activation here
        # already waits on a DVE-produced input).
        pact = cpool.tile([BS, 4], f32)
        nc.vector.tensor_copy(pact[:], ptile[:])
        # out column 0 is SOC_init itself
        nc.scalar.dma_start(od[:, 0:1], ptile[:, 0:1])

        sizes = []
        rem = T - 1
        while rem > 0:
            sizes.append(min(TC, rem))
            rem -= sizes[-1]

        for _rep in range(reps):
            carry = ptile[:, 0:1]
            s = 0
            for L in sizes:
                xt = xpool.tile([BS, (TC + 1) * F], f32)
                nc.sync.dma_start(
                    xt[:, : (L + 1) * F], xd[:, s * F : (s + L + 1) * F]
                )
                x3 = xt[:].rearrange("p (t f) -> p t f", f=F)

                ts0 = x3[:, 0:L, 0]
                ts1 = x3[:, 1 : L + 1, 0]
                cur = x3[:, 0:L, 1]
                piv = x3[:, 0:L, piv_col]
                oth = x3[:, 0:L, oth_col]

                # dt = ts[t+1] - ts[t] into its own contiguous tile (a strided
                # scratch write into the x tile costs ~25% more DVE time and
                # makes the next mul's read strided too)
                dtt = dpool.tile([BS, TC], f32)
                nc.vector.tensor_sub(dtt[:, :L], ts1, ts0)
                wt = wpool.tile([BS, TC], f32)
                # wt = piv*k + oth   (the softplus pre-activation, un-scaled)
                nc.vector.scalar_tensor_tensor(
                    wt[:, :L], piv, float(k_piv), oth,
                    mybir.AluOpType.mult, mybir.AluOpType.add,
                )
                # wt = softplus(act_scale*wt + act_bias) = ln(1 + exp(.))
                # (the toolchain's ACT func sets have exp+ln+identity together;
                #  native Softplus fails to lower)
                nc.scalar.activation(
                    wt[:, :L], wt[:, :L], mybir.ActivationFunctionType.Exp,
                    bias=pact[:, 3:4], scale=float(act_scale),
                )
                nc.scalar.activation(
                    wt[:, :L], wt[:, :L], mybir.ActivationFunctionType.Ln,
                    bias=1.0, scale=1.0,
                )
                # wt = q2*wt + q1    (per-cell scalars)
                nc.scalar.activation(
                    wt[:, :L], wt[:, :L], mybir.ActivationFunctionType.Identity,
                    bias=pact[:, 1:2], scale=pact[:, 2:3],
                )
                mt = mpool.tile([BS, TC], f32)
                nc.vector.tensor_mul(mt[:, :L], cur, dtt[:, :L])
                et = epool.tile([BS, TC], f32)
                nc.vector.tensor_mul(et[:, :L], mt[:, :L], wt[:, :L])
                # running SOC: r[i] = carry + cumsum(incr)[i]
                rt = rpool.tile([BS, TC], f32)
                nc.vector.tensor_tensor_scan(
                    rt[:, :L], ones[:, :L], et[:, :L], carry,
                    mybir.AluOpType.mult, mybir.AluOpType.add,
                )
                # output DMA on the scalar engine's HWDGE ring: keeps the
                # GPSIMD Q7 free and off the output path entirely
                nc.scalar.dma_start(od[:, s + 1 : s + L + 1], rt[:, :L])
                carry = rt[:, L - 1 : L]
                s += L

    # neuronxcc codegen allows at most one sync wait per instruction; split
    # multi-wait instructions the way Bacc.compile() would.
    _bass_rust.generate_event_semaphores(nc)
    return nc


def _prep(X, SC, W1i, b1i, W2i, b2i, W1e, b1e, W2e, b2e):
    """Host precompute: returns (build_params, in_maps) where
    build_params = (k_piv, piv_col, oth_col, act_scale) for _build_program
    and in_maps is the per-core input dict list."""
    X = np.ascontiguousarray(np.asarray(X), dtype=np.float32)
    SC = np.ascontiguousarray(np.asarray(SC), dtype=np.float32)
    W1i = np.asarray(W1i, dtype=np.float64)
    b1i = np.asarray(b1i, dtype=np.float64)
    W2i = np.asarray(W2i, dtype=np.float64)
    b2i = np.asarray(b2i, dtype=np.float64)
    W1e = np.asarray(W1e, dtype=np.float64)
    b1e = np.asarray(b1e, dtype=np.float64)
    W2e = np.asarray(W2e, dtype=np.float64)
    b2e = np.asarray(b2e, dtype=np.float64)

    # ---- host precompute of tiny per-cell scalars (float64 for accuracy) ----
    Q = SC[:, 0].astype(np.float64)
    eta0 = SC[:, 1].astype(np.float64)
    R = SC[:, 2].astype(np.float64)
    soc_base = SC[:, 3].astype(np.float64)

    feat0 = np.stack(
        [X[:, 0, 1], X[:, 0, 2], X[:, 0, 3], SC[:, 2]], axis=-1
    ).astype(np.float64)  # [B, 4] = (I0, Temp0, U0, R)
    z = feat0 @ W1i.T + b1i
    h0 = _softplus64(z)
    soc_net = (h0 @ W2i.T + b2i)[:, 0]
    soc_init = soc_base * (1.0 + soc_net)  # [B]

    c = eta0 / (3600.0 * Q)
    b2e_f = float(np.asarray(b2e).reshape(-1)[0])
    w2e_f = float(np.asarray(W2e).reshape(-1)[0])
    q1 = c * (1.0 + b2e_f)  # [B]
    q2 = c * w2e_f          # [B]

    # pre-activation a = w0*I + w1*Temp + b1e, computed as
    # act_scale*(piv*k + oth) + act_bias with the larger weight as pivot
    w0 = float(np.asarray(W1e).reshape(-1)[0])
    w1 = float(np.asarray(W1e).reshape(-1)[1])
    b1e_f = float(np.asarray(b1e).reshape(-1)[0])
    if abs(w0) >= abs(w1):
        # a = w0*(I + (w1/w0)*Temp) + b  -> pivot=Temp(col2), other=I(col1)
        piv_col, oth_col = 2, 1
        k_piv = w1 / w0 if w0 != 0.0 else 0.0
        act_scale = w0
    else:
        piv_col, oth_col = 1, 2
        k_piv = w0 / w1
        act_scale = w1

    P = np.stack(
        [soc_init, q1, q2, np.full_like(q1, b1e_f)], axis=-1
    ).astype(np.float32)  # [B, 4]

    in_maps = []
    for ci in range(NCORES):
        sl = slice(ci * BS, (ci + 1) * BS)
        in_maps.append(
            {
                "x": np.ascontiguousarray(X[sl]).reshape(BS, T * F),
                "p": np.ascontiguousarray(P[sl]),
            }
        )
    return (k_piv, piv_col, oth_col, act_scale), in_maps


def kernel(X, SC, W1i, b1i, W2i, b2i, W1e, b1e, W2e, b2e):
    from concourse.bass_utils import run_bass_kernel_spmd

    params, in_maps = _prep(X, SC, W1i, b1i, W2i, b2i, W1e, b1e, W2e, b2e)
    nc = _build_program(*params)

    res = run_bass_kernel_spmd(nc, in_maps, list(range(NCORES)))
    out = np.concatenate([res.results[ci]["o"] for ci in range(NCORES)], axis=0)
    return out.reshape(B, T, 1)

